# revision 17
# baseline (speedup 1.0000x reference)
"""Trainium2 Bass kernel for nn_CorrFusion (pairwise-MLP correlation + PointNet).

Math (B=1, N=1024, C=64):
  The pairwise layer-1 MLP on concat(f1_i, f2_j) is separable:
      h1[:, i, j] = relu(A[:, i] + Bq[:, j])
  with A = W1_0[:, :64] @ F1  and  Bq = W1_0[:, 64:] @ F2 + b1_0.
  So the N x N x 2C concat tensor is never materialized. The expensive part is
      hmax[:, j] = relu(max_i(W1_1 @ h1[:, :, j]) + b1_1)
  (relu/bias commute with the max over i).

Sharding: j (columns of pc_2) is split across the 8 cores, 128 j's per core.
The max over i is then a purely local free-axis reduction - no collectives.
Each core processes two j's at a time stacked in the partition dim:
  ACT:  t = relu(A_stack + bm[:, q])            [128, 1024] SBUF (fp32r)
  PE :  ps chunk = blockdiag(W1_1, W1_1) @ t    [128, 2048] PSUM (2 passes)
  DVE:  hm[:, 2s:2s+2] = max over i of ps       [128, 2]
Tail (mlp_2 + PointNet on the local j-shard) is tiny. The global feature
(gfeat, max over all points of h3) is combined on the host from the 8
per-core partial maxima during unsharding.

Engine budget per core (measured): ACT 64 relu ops ~73us, DVE 32 double
reduces ~73us, PE 128 fp32r matmuls ~60us - ACT/DVE are the joint wall.
GPSIMD was measured at 14.7us per [128,1024] tensor_scalar (generic ucode
path, ~17 cyc/elem) so no elementwise work is placed there.
"""

import numpy as np
from contextlib import ExitStack

import concourse.bass as bass  # noqa: F401
import concourse.tile as tile
from concourse import bacc, mybir
from concourse.bass_utils import run_bass_kernel_spmd

N = 1024
C = 64
NCORES = 8
JS = N // NCORES      # j's per core (128)
Q = JS // 2           # relu passes per core, 2 j's each (64)
S = Q // 2            # psum super-passes, 2 relu tiles each (32)

F32 = mybir.dt.float32
F32R = mybir.dt.float32r
AF = mybir.ActivationFunctionType
ALU = mybir.AluOpType
AX = mybir.AxisListType

TRACE = False
LAST_RESULT = None

_nc_cache = None

# packed-constant column layouts: (name, n_rows, n_cols)
PACKA1 = [("w10lt2", C, 128), ("w10rt2", C, 128), ("f2s", C, JS),
          ("b10s", 128, 1)]
PACKA2 = [("bd", 128, 128), ("b11s", 128, 1)]
PACKB = [("w20t", C, C), ("w21t", C, 3), ("wp1t", 3, C), ("wp2t", C, 128),
         ("pc1b", 3, JS), ("b20", C, 1), ("b21", 3, 1), ("bp1c", C, 1),
         ("bp2c", 128, 1), ("bp3t", 128, 8)]
XA1 = sum(c for _, _, c in PACKA1)
XA2 = sum(c for _, _, c in PACKA2)
XB = sum(c for _, _, c in PACKB)


def _slices(pack):
    out = {}
    col = 0
    for name, rows, cols in pack:
        out[name] = (rows, col, cols)
        col += cols
    return out


SLA1 = _slices(PACKA1)
SLA2 = _slices(PACKA2)
SLB = _slices(PACKB)


def _emit(tc, nc, io):
    with ExitStack() as ctx:
        singles = ctx.enter_context(tc.tile_pool(name="singles", bufs=1))
        relu_pool = ctx.enter_context(tc.tile_pool(name="relu", bufs=6))

        # ---- input DMAs: f1 quarters (sync, parallel queues), packed consts
        # (gpsimd so the Sync engine's serial ~0.6us per-DMA issue cost
        # doesn't stack), wp3t last (only needed by the tail).
        f1 = singles.tile([C, N], F32, tag="f1", name="f1_sb")
        for qq in range(4):
            nc.sync.dma_start(out=f1[:, qq * 256:(qq + 1) * 256],
                              in_=io["f1"][:, qq * 256:(qq + 1) * 256])
        pa1 = singles.tile([128, XA1], F32, tag="pa1", name="pa1_sb")
        nc.gpsimd.dma_start(out=pa1[:, :], in_=io["packa1"][:, :])
        pa2 = singles.tile([128, XA2], F32, tag="pa2", name="pa2_sb")
        nc.gpsimd.dma_start(out=pa2[:, :], in_=io["packa2"][:, :])
        pb = singles.tile([128, XB], F32, tag="pb", name="pb_sb")
        nc.gpsimd.dma_start(out=pb[:, :], in_=io["packb"][:, :])
        wp3t = singles.tile([128, N], F32, tag="wp3t", name="wp3t_sb")
        nc.sync.dma_start(out=wp3t[:, :], in_=io["wp3t"][:, :])

        def sl(tile_, table, name):
            rows, col, cols = table[name]
            return tile_[0:rows, col:col + cols]

        w10lt2 = sl(pa1, SLA1, "w10lt2")
        w10rt2 = sl(pa1, SLA1, "w10rt2")
        f2s = sl(pa1, SLA1, "f2s")
        b10s = sl(pa1, SLA1, "b10s")
        bd = sl(pa2, SLA2, "bd")
        b11s = sl(pa2, SLA2, "b11s")
        w20t = sl(pb, SLB, "w20t")
        w21t = sl(pb, SLB, "w21t")
        wp1t = sl(pb, SLB, "wp1t")
        wp2t = sl(pb, SLB, "wp2t")
        pc1b = sl(pb, SLB, "pc1b")
        b20 = sl(pb, SLB, "b20")
        b21 = sl(pb, SLB, "b21")
        bp1 = sl(pb, SLB, "bp1c")
        bp2 = sl(pb, SLB, "bp2c")
        bp3t = sl(pb, SLB, "bp3t")

        # fp32r operands must be produced by a compute op (which rounds); a
        # raw DMA-loaded fp32r operand crashes the PE (EXEC_UNIT_UNRECOVERABLE).
        # All rounding copies run on DVE, which is idle until the first
        # loop reduce.
        bdr = singles.tile([128, 128], F32R, tag="bdr", name="bdr")
        nc.vector.tensor_copy(bdr[:, :], bd)
        w21tr = singles.tile([C, 3], F32R, tag="w21tr", name="w21tr")
        wp1tr = singles.tile([3, C], F32R, tag="wp1tr", name="wp1tr")
        wp2tr = singles.tile([C, 128], F32R, tag="wp2tr", name="wp2tr")
        wp3r = singles.tile([128, N], F32R, tag="wp3r", name="wp3r")

        a_st = singles.tile([128, N], F32, tag="a_st", name="a_st")
        bm = singles.tile([128, Q], F32, tag="bm", name="bm")
        hm = singles.tile([128, Q], F32, tag="hm", name="hm")

        with tc.tile_pool(name="psA", bufs=2, space="PSUM") as psA:
            # A_stack[c + 64*s, i] = (W1_0[:, :64] @ F1)[c, i]  for s in {0,1}
            a_ps = psA.tile([128, N], F32, tag="ps", name="a_ps")
            for h in range(4):
                nc.tensor.matmul(
                    a_ps[:, h * 256:(h + 1) * 256],
                    w10lt2,
                    f1[:, h * 256:(h + 1) * 256],
                    start=True, stop=True,
                )
            nc.vector.tensor_copy(a_st[:, :], a_ps[:, :])

            # Bq stacked: rows 0-63 and 64-127 both = W1_0[:, 64:] @ F2_shard;
            # the b1_0 bias-add writes directly into the two bm halves
            # (col q of bm is [Bq[:, q] ; Bq[:, 64+q]]).
            bq_ps = psA.tile([128, JS], F32, tag="ps", name="bq_ps")
            nc.tensor.matmul(bq_ps[:, :], w10rt2, f2s, start=True, stop=True)
            nc.scalar.activation(bm[0:C, :], bq_ps[0:C, 0:Q], func=AF.Identity,
                                 bias=b10s[0:C, 0:1], scale=1.0)
            nc.scalar.activation(bm[C:128, :], bq_ps[C:128, Q:JS],
                                 func=AF.Identity, bias=b10s[C:128, 0:1],
                                 scale=1.0)

            for s in range(S):
                ps = psA.tile([128, 2 * N], F32, tag="ps", name=f"ps{s}")
                for half in range(2):
                    q = 2 * s + half
                    t = relu_pool.tile([128, N], F32R, tag="t", name=f"t{q}")
                    nc.scalar.activation(t[:, :], a_st[:, :], func=AF.Relu,
                                         bias=bm[:, q:q + 1], scale=1.0)
                    for h in range(2):
                        nc.tensor.matmul(
                            ps[:, half * N + h * 512:half * N + (h + 1) * 512],
                            bdr[:, :],
                            t[:, h * 512:(h + 1) * 512],
                            start=True, stop=True,
                        )
                nc.vector.tensor_reduce(
                    out=hm[:, 2 * s:2 * s + 2],
                    in_=ps[:, :].rearrange("p (n x) -> p n x", n=2),
                    axis=AX.X, op=ALU.max)
                if s == 0:
                    # tail-weight fp32r rounding copies: DVE work hidden in
                    # the loop (they only gate the tail)
                    nc.vector.tensor_copy(w21tr[:, :], w21t)
                    nc.vector.tensor_copy(wp1tr[:, :], wp1t)
                    nc.vector.tensor_copy(wp2tr[:, :], wp2t)
                    nc.vector.tensor_copy(wp3r[:, :], wp3t[:, :])

        # ---- tail: mlp_2 + PointNet on the local shard ----
        # hmax cols 0-63 live in partitions 0-63 of hm, cols 64-127 in
        # partitions 64-127; compute-engine lanes cannot cross partitions, so
        # the bottom half goes through a small SBUF->SBUF DMA.
        hm64 = singles.tile([C, JS], F32, tag="hm64", name="hm64")
        hmrB = singles.tile([128, Q], F32, tag="hmrB", name="hmrB")
        nc.scalar.activation(hmrB[C:128, :], hm[C:128, :], func=AF.Relu,
                             bias=b11s[C:128, 0:1], scale=1.0)
        nc.sync.dma_start(out=hm64[:, Q:JS], in_=hmrB[C:128, :])
        nc.scalar.activation(hm64[:, 0:Q], hm[0:C, :], func=AF.Relu,
                             bias=b11s[0:C, 0:1], scale=1.0)

        with tc.tile_pool(name="psT", bufs=4, space="PSUM") as psT:
            # split so the top half runs while the hm64 bottom-half DMA flies
            m_ps = psT.tile([C, JS], F32, tag="tps", name="m_ps")
            nc.tensor.matmul(m_ps[:, 0:Q], w20t, hm64[:, 0:Q],
                             start=True, stop=True)
            nc.tensor.matmul(m_ps[:, Q:JS], w20t, hm64[:, Q:JS],
                             start=True, stop=True)
            m_sb = singles.tile([C, JS], F32R, tag="m_sb", name="m_sb")
            nc.scalar.activation(m_sb[:, :], m_ps[:, :], func=AF.Relu,
                                 bias=b20[:, 0:1], scale=1.0)

            ff_ps = psT.tile([3, JS], F32, tag="tps", name="ff_ps")
            nc.tensor.matmul(ff_ps[:, :], w21tr[:, :], m_sb[:, :],
                             start=True, stop=True)
            # critical path: x = (ff + b2_1) + pc1.T = ff_psum + pc1b in one
            # DVE op straight from PSUM; the biased copy for the ffo output
            # happens off the critical path on ACT.
            x = singles.tile([3, JS], F32R, tag="x", name="x")
            nc.vector.tensor_add(x[:, :], ff_ps[:, :], pc1b)
            ffT = singles.tile([3, JS], F32, tag="ffT", name="ffT")
            nc.scalar.activation(ffT[:, :], ff_ps[:, :], func=AF.Identity,
                                 bias=b21[:, 0:1], scale=1.0)
            nc.sync.dma_start(out=io["ffo"][:, :], in_=ffT[:, :])

            pf_ps = psT.tile([C, JS], F32, tag="tps", name="pf_ps")
            nc.tensor.matmul(pf_ps[:, :], wp1tr[:, :], x[:, :],
                             start=True, stop=True)
            pf = singles.tile([C, JS], F32R, tag="pf", name="pf")
            nc.scalar.activation(pf[:, :], pf_ps[:, :], func=AF.Relu,
                                 bias=bp1[:, 0:1], scale=1.0)
            nc.sync.dma_start(out=io["pfo"][:, :], in_=pf[:, :].bitcast(F32))

            h2_ps = psT.tile([128, JS], F32, tag="tps", name="h2_ps")
            nc.tensor.matmul(h2_ps[:, :], wp2tr[:, :], pf[:, :],
                             start=True, stop=True)
            h2t = singles.tile([128, JS], F32R, tag="h2t", name="h2t")
            nc.scalar.activation(h2t[:, :], h2_ps[:, :], func=AF.Relu,
                                 bias=bp2[:, 0:1], scale=1.0)

            # h3 = Wp3 @ h2 in 8 output-channel chunks into one [128, 1024]
            # PSUM tile, then a single 8-segment max-reduce -> [128, 8].
            h3_ps = psT.tile([128, N], F32, tag="h3ps", name="h3_ps", bufs=1)
            for m2 in range(8):
                nc.tensor.matmul(h3_ps[:, m2 * 128:(m2 + 1) * 128],
                                 wp3r[:, m2 * 128:(m2 + 1) * 128],
                                 h2t[:, :], start=True, stop=True)
            g = singles.tile([128, 8], F32, tag="g", name="g")
            nc.vector.tensor_reduce(
                out=g[:, :], in_=h3_ps[:, :].rearrange("p (n x) -> p n x", n=8),
                axis=AX.X, op=ALU.max)
            gb = singles.tile([128, 8], F32, tag="gb", name="gb")
            nc.vector.tensor_add(gb[:, :], g[:, :], bp3t)
            nc.sync.dma_start(out=io["g3o"][:, :], in_=gb[:, :])


def _build():
    global _nc_cache
    if _nc_cache is not None:
        return _nc_cache
    nc = bacc.Bacc(trn_type="TRN2")

    io = dict(
        f1=nc.dram_tensor("f1", [C, N], F32, kind="ExternalInput"),
        packa1=nc.dram_tensor("packa1", [128, XA1], F32, kind="ExternalInput"),
        packa2=nc.dram_tensor("packa2", [128, XA2], F32, kind="ExternalInput"),
        packb=nc.dram_tensor("packb", [128, XB], F32, kind="ExternalInput"),
        wp3t=nc.dram_tensor("wp3t", [128, N], F32, kind="ExternalInput"),
        ffo=nc.dram_tensor("ffo", [3, JS], F32, kind="ExternalOutput"),
        pfo=nc.dram_tensor("pfo", [C, JS], F32, kind="ExternalOutput"),
        g3o=nc.dram_tensor("g3o", [128, 8], F32, kind="ExternalOutput"),
    )
    with tile.TileContext(nc) as tc:
        _emit(tc, nc, io)
    nc.finalize()
    _nc_cache = nc
    return nc


def _pack(table, fields):
    cols = sum(c for _, _, c in table)
    out = np.zeros((128, cols), np.float32)
    col = 0
    for name, rows, ncols in table:
        v = fields[name]
        assert v.shape == (rows, ncols), (name, v.shape, rows, ncols)
        out[:rows, col:col + ncols] = v
        col += ncols
    return out


def kernel(**inputs):
    global LAST_RESULT
    nc = _build()
    a = {k: np.ascontiguousarray(np.asarray(v, dtype=np.float32))
         for k, v in inputs.items()}
    c = np.ascontiguousarray

    pc_1 = a["pc_1"][0]          # [N, 3]
    F1 = a["pc_feat_1"][0]       # [C, N]
    F2 = a["pc_feat_2"][0]       # [C, N]
    W1_0, b1_0 = a["W1_0"], a["b1_0"]
    W1_1, b1_1 = a["W1_1"], a["b1_1"]
    W2_0, b2_0 = a["W2_0"], a["b2_0"]
    W2_1, b2_1 = a["W2_1"], a["b2_1"]
    Wp1, bp1 = a["Wp1"], a["bp1"]
    Wp2, bp2 = a["Wp2"], a["bp2"]
    Wp3, bp3 = a["Wp3"], a["bp3"]

    bdm = np.zeros((128, 128), np.float32)
    bdm[:C, :C] = W1_1.T
    bdm[C:, C:] = W1_1.T

    sharedA = dict(
        w10lt2=np.concatenate([W1_0[:, :C].T, W1_0[:, :C].T], axis=1),
        w10rt2=np.concatenate([W1_0[:, C:].T, W1_0[:, C:].T], axis=1),
        b10s=np.concatenate([b1_0, b1_0])[:, None],
        bd=bdm,
        b11s=np.concatenate([b1_1, b1_1])[:, None],
    )
    sharedB = dict(
        w20t=W2_0.T, w21t=W2_1.T, wp1t=Wp1.T, wp2t=Wp2.T,
        b20=b2_0[:, None], b21=b2_1[:, None], bp1c=bp1[:, None],
        bp2c=bp2[:, None], bp3t=bp3.reshape(8, 128).T,
    )
    wp3t = c(Wp3.T)

    in_maps = []
    for k in range(NCORES):
        sl = slice(k * JS, (k + 1) * JS)
        fa = dict(sharedA)
        fa["f2s"] = F2[:, sl]
        fb = dict(sharedB)
        fb["pc1b"] = pc_1[sl, :].T + b2_1[:, None]
        in_maps.append(dict(
            f1=F1,
            packa1=_pack(PACKA1, fa),
            packa2=_pack(PACKA2, fa),
            packb=_pack(PACKB, fb),
            wp3t=wp3t,
        ))

    res = run_bass_kernel_spmd(nc, in_maps, core_ids=list(range(NCORES)),
                               trace=TRACE)
    LAST_RESULT = res
    outs = res.results

    ff = np.concatenate([outs[k]["ffo"] for k in range(NCORES)], axis=1)[None]
    pf_all = np.concatenate([outs[k]["pfo"].T for k in range(NCORES)], axis=0)
    g = np.max(np.stack([outs[k]["g3o"].T.reshape(N) for k in range(NCORES)]),
               axis=0)
    fc = np.concatenate([np.broadcast_to(g, (N, N)), pf_all], axis=1)[None]
    return c(ff.astype(np.float32)), c(fc.astype(np.float32))


# revision 19
# speedup vs baseline: 1.0323x; 1.0323x over previous
"""Trainium2 Bass kernel for nn_CorrFusion (pairwise-MLP correlation + PointNet).

Math (B=1, N=1024, C=64):
  The pairwise layer-1 MLP on concat(f1_i, f2_j) is separable:
      h1[:, i, j] = relu(A[:, i] + Bq[:, j])
  with A = W1_0[:, :64] @ F1  and  Bq = W1_0[:, 64:] @ F2 + b1_0.
  So the N x N x 2C concat tensor is never materialized. The expensive part is
      hmax[:, j] = relu(max_i(W1_1 @ h1[:, :, j]) + b1_1)
  (relu/bias commute with the max over i).

Sharding: j (columns of pc_2) is split across the 8 cores, 128 j's per core.
The max over i is then a purely local free-axis reduction - no collectives.
Each core processes two j's at a time stacked in the partition dim:
  ACT:  t = relu(A_stack + bm[:, q])            [128, 1024] SBUF (fp32r)
  PE :  ps chunk = blockdiag(W1_1, W1_1) @ t    [128, 2048] PSUM (2 passes)
  DVE:  hm[:, 2s:2s+2] = max over i of ps       [128, 2]
Tail (mlp_2 + PointNet on the local j-shard) is tiny. The global feature
(gfeat, max over all points of h3) is combined on the host from the 8
per-core partial maxima during unsharding.

Engine budget per core (measured): ACT 64 relu ops ~73us, DVE 32 double
reduces ~73us, PE 128 fp32r matmuls ~60us - ACT/DVE are the joint wall.
GPSIMD was measured at 14.7us per [128,1024] tensor_scalar (generic ucode
path, ~17 cyc/elem) so no elementwise work is placed there.
"""

import numpy as np
from contextlib import ExitStack

import concourse.bass as bass  # noqa: F401
import concourse.tile as tile
from concourse import bacc, mybir
from concourse.bass_utils import run_bass_kernel_spmd

N = 1024
C = 64
NCORES = 8
JS = N // NCORES      # j's per core (128)
Q = JS // 2           # relu passes per core, 2 j's each (64)
S = Q // 2            # psum super-passes, 2 relu tiles each (32)

F32 = mybir.dt.float32
F32R = mybir.dt.float32r
AF = mybir.ActivationFunctionType
ALU = mybir.AluOpType
AX = mybir.AxisListType

TRACE = False
LAST_RESULT = None

_nc_cache = None

# packed-constant column layouts: (name, n_rows, n_cols)
PACKA1 = [("w10lt2", C, 128), ("w10rt2", C, 128), ("f2s", C, JS),
          ("b10s", 128, 1)]
PACKA2 = [("bd", 128, 128), ("b11s", 128, 1)]
PACKB = [("w20t", C, C), ("w21t", C, 3), ("wp1t", 3, C), ("wp2t", C, 128),
         ("pc1b", 3, JS), ("b20", C, 1), ("b21", 3, 1), ("bp1c", C, 1),
         ("bp2c", 128, 1), ("bp3t", 128, 8)]
XA1 = sum(c for _, _, c in PACKA1)
XA2 = sum(c for _, _, c in PACKA2)
XB = sum(c for _, _, c in PACKB)


def _slices(pack):
    out = {}
    col = 0
    for name, rows, cols in pack:
        out[name] = (rows, col, cols)
        col += cols
    return out


SLA1 = _slices(PACKA1)
SLA2 = _slices(PACKA2)
SLB = _slices(PACKB)


def _emit(tc, nc, io):
    with ExitStack() as ctx:
        singles = ctx.enter_context(tc.tile_pool(name="singles", bufs=1))
        relu_pool = ctx.enter_context(tc.tile_pool(name="relu", bufs=6))

        # ---- input DMAs. The per-dma_start trigger costs ~0.6us of engine
        # time, so issues are spread over idle engines and ordered by
        # criticality: packa1 (weights for the head matmuls) first on gpsimd,
        # f1 quarters across sync/scalar/vector, then packa2/packb/wp3t.
        f1 = singles.tile([C, N], F32, tag="f1", name="f1_sb")
        pa1 = singles.tile([128, XA1], F32, tag="pa1", name="pa1_sb")
        nc.gpsimd.dma_start(out=pa1[:, :], in_=io["packa1"][:, :])
        f1_eng = [nc.sync, nc.scalar, nc.sync, nc.scalar]
        for qq in range(4):
            f1_eng[qq].dma_start(out=f1[:, qq * 256:(qq + 1) * 256],
                                 in_=io["f1"][:, qq * 256:(qq + 1) * 256])
        pa2 = singles.tile([128, XA2], F32, tag="pa2", name="pa2_sb")
        nc.gpsimd.dma_start(out=pa2[:, :], in_=io["packa2"][:, :])
        pb = singles.tile([128, XB], F32, tag="pb", name="pb_sb")
        nc.gpsimd.dma_start(out=pb[:, :], in_=io["packb"][:, :])
        wp3t = singles.tile([128, N], F32, tag="wp3t", name="wp3t_sb")
        nc.sync.dma_start(out=wp3t[:, :], in_=io["wp3t"][:, :])

        def sl(tile_, table, name):
            rows, col, cols = table[name]
            return tile_[0:rows, col:col + cols]

        w10lt2 = sl(pa1, SLA1, "w10lt2")
        w10rt2 = sl(pa1, SLA1, "w10rt2")
        f2s = sl(pa1, SLA1, "f2s")
        b10s = sl(pa1, SLA1, "b10s")
        bd = sl(pa2, SLA2, "bd")
        b11s = sl(pa2, SLA2, "b11s")
        w20t = sl(pb, SLB, "w20t")
        w21t = sl(pb, SLB, "w21t")
        wp1t = sl(pb, SLB, "wp1t")
        wp2t = sl(pb, SLB, "wp2t")
        pc1b = sl(pb, SLB, "pc1b")
        b20 = sl(pb, SLB, "b20")
        b21 = sl(pb, SLB, "b21")
        bp1 = sl(pb, SLB, "bp1c")
        bp2 = sl(pb, SLB, "bp2c")
        bp3t = sl(pb, SLB, "bp3t")

        # fp32r operands must be produced by a compute op (which rounds); a
        # raw DMA-loaded fp32r operand crashes the PE (EXEC_UNIT_UNRECOVERABLE).
        # All rounding copies run on DVE, which is idle until the first
        # loop reduce.
        w10lt2r = singles.tile([C, 128], F32R, tag="w10lt2r", name="w10lt2r")
        nc.vector.tensor_copy(w10lt2r[:, :], w10lt2)
        f1r = singles.tile([C, N], F32R, tag="f1r", name="f1r")
        for qq in range(4):
            nc.vector.tensor_copy(f1r[:, qq * 256:(qq + 1) * 256],
                                  f1[:, qq * 256:(qq + 1) * 256])
        bdr = singles.tile([128, 128], F32R, tag="bdr", name="bdr")
        nc.vector.tensor_copy(bdr[:, :], bd)
        w21tr = singles.tile([C, 3], F32R, tag="w21tr", name="w21tr")
        wp1tr = singles.tile([3, C], F32R, tag="wp1tr", name="wp1tr")
        wp2tr = singles.tile([C, 128], F32R, tag="wp2tr", name="wp2tr")
        wp3r = singles.tile([128, N], F32R, tag="wp3r", name="wp3r")

        a_st = singles.tile([128, N], F32, tag="a_st", name="a_st")
        bm = singles.tile([128, Q], F32, tag="bm", name="bm")
        hm = singles.tile([128, Q], F32, tag="hm", name="hm")

        with tc.tile_pool(name="psA", bufs=2, space="PSUM") as psA:
            # A_stack[c + 64*s, i] = (W1_0[:, :64] @ F1)[c, i]  for s in {0,1}
            a_ps = psA.tile([128, N], F32, tag="ps", name="a_ps")
            for h in range(4):
                nc.tensor.matmul(
                    a_ps[:, h * 256:(h + 1) * 256],
                    w10lt2r[:, :],
                    f1r[:, h * 256:(h + 1) * 256],
                    start=True, stop=True,
                )
            nc.vector.tensor_copy(a_st[:, :], a_ps[:, :])

            # Bq stacked: rows 0-63 and 64-127 both = W1_0[:, 64:] @ F2_shard;
            # the b1_0 bias-add writes directly into the two bm halves
            # (col q of bm is [Bq[:, q] ; Bq[:, 64+q]]).
            bq_ps = psA.tile([128, JS], F32, tag="ps", name="bq_ps")
            nc.tensor.matmul(bq_ps[:, :], w10rt2, f2s, start=True, stop=True)
            nc.scalar.activation(bm[0:C, :], bq_ps[0:C, 0:Q], func=AF.Identity,
                                 bias=b10s[0:C, 0:1], scale=1.0)
            nc.scalar.activation(bm[C:128, :], bq_ps[C:128, Q:JS],
                                 func=AF.Identity, bias=b10s[C:128, 0:1],
                                 scale=1.0)

            for s in range(S):
                ps = psA.tile([128, 2 * N], F32, tag="ps", name=f"ps{s}")
                for half in range(2):
                    q = 2 * s + half
                    t = relu_pool.tile([128, N], F32R, tag="t", name=f"t{q}")
                    nc.scalar.activation(t[:, :], a_st[:, :], func=AF.Relu,
                                         bias=bm[:, q:q + 1], scale=1.0)
                    for h in range(2):
                        nc.tensor.matmul(
                            ps[:, half * N + h * 512:half * N + (h + 1) * 512],
                            bdr[:, :],
                            t[:, h * 512:(h + 1) * 512],
                            start=True, stop=True,
                        )
                nc.vector.tensor_reduce(
                    out=hm[:, 2 * s:2 * s + 2],
                    in_=ps[:, :].rearrange("p (n x) -> p n x", n=2),
                    axis=AX.X, op=ALU.max)
                if s == 0:
                    # tail-weight fp32r rounding copies: DVE work hidden in
                    # the loop (they only gate the tail)
                    nc.vector.tensor_copy(w21tr[:, :], w21t)
                    nc.vector.tensor_copy(wp1tr[:, :], wp1t)
                    nc.vector.tensor_copy(wp2tr[:, :], wp2t)
                    nc.vector.tensor_copy(wp3r[:, :], wp3t[:, :])

        # ---- tail: mlp_2 + PointNet on the local shard ----
        # hmax cols 0-63 live in partitions 0-63 of hm, cols 64-127 in
        # partitions 64-127; compute-engine lanes cannot cross partitions, so
        # the bottom half goes through a small SBUF->SBUF DMA.
        hm64 = singles.tile([C, JS], F32, tag="hm64", name="hm64")
        hmrB = singles.tile([128, Q], F32, tag="hmrB", name="hmrB")
        nc.scalar.activation(hmrB[C:128, :], hm[C:128, :], func=AF.Relu,
                             bias=b11s[C:128, 0:1], scale=1.0)
        nc.sync.dma_start(out=hm64[:, Q:JS], in_=hmrB[C:128, :])
        nc.scalar.activation(hm64[:, 0:Q], hm[0:C, :], func=AF.Relu,
                             bias=b11s[0:C, 0:1], scale=1.0)

        with tc.tile_pool(name="psT", bufs=4, space="PSUM") as psT:
            # split so the top half runs while the hm64 bottom-half DMA flies
            m_ps = psT.tile([C, JS], F32, tag="tps", name="m_ps")
            nc.tensor.matmul(m_ps[:, 0:Q], w20t, hm64[:, 0:Q],
                             start=True, stop=True)
            nc.tensor.matmul(m_ps[:, Q:JS], w20t, hm64[:, Q:JS],
                             start=True, stop=True)
            m_sb = singles.tile([C, JS], F32R, tag="m_sb", name="m_sb")
            nc.scalar.activation(m_sb[:, :], m_ps[:, :], func=AF.Relu,
                                 bias=b20[:, 0:1], scale=1.0)

            ff_ps = psT.tile([3, JS], F32, tag="tps", name="ff_ps")
            nc.tensor.matmul(ff_ps[:, :], w21tr[:, :], m_sb[:, :],
                             start=True, stop=True)
            # critical path: x = (ff + b2_1) + pc1.T = ff_psum + pc1b in one
            # DVE op straight from PSUM; the biased copy for the ffo output
            # happens off the critical path on ACT.
            x = singles.tile([3, JS], F32R, tag="x", name="x")
            nc.vector.tensor_add(x[:, :], ff_ps[:, :], pc1b)
            ffT = singles.tile([3, JS], F32, tag="ffT", name="ffT")
            nc.scalar.activation(ffT[:, :], ff_ps[:, :], func=AF.Identity,
                                 bias=b21[:, 0:1], scale=1.0)
            nc.sync.dma_start(out=io["ffo"][:, :], in_=ffT[:, :])

            pf_ps = psT.tile([C, JS], F32, tag="tps", name="pf_ps")
            nc.tensor.matmul(pf_ps[:, :], wp1tr[:, :], x[:, :],
                             start=True, stop=True)
            pf = singles.tile([C, JS], F32R, tag="pf", name="pf")
            nc.scalar.activation(pf[:, :], pf_ps[:, :], func=AF.Relu,
                                 bias=bp1[:, 0:1], scale=1.0)
            nc.sync.dma_start(out=io["pfo"][:, :], in_=pf[:, :].bitcast(F32))

            h2_ps = psT.tile([128, JS], F32, tag="tps", name="h2_ps")
            nc.tensor.matmul(h2_ps[:, :], wp2tr[:, :], pf[:, :],
                             start=True, stop=True)
            h2t = singles.tile([128, JS], F32R, tag="h2t", name="h2t")
            nc.scalar.activation(h2t[:, :], h2_ps[:, :], func=AF.Relu,
                                 bias=bp2[:, 0:1], scale=1.0)

            # h3 = Wp3 @ h2 in 8 output-channel chunks into one [128, 1024]
            # PSUM tile, then a single 8-segment max-reduce -> [128, 8].
            h3_ps = psT.tile([128, N], F32, tag="h3ps", name="h3_ps", bufs=1)
            for m2 in range(8):
                nc.tensor.matmul(h3_ps[:, m2 * 128:(m2 + 1) * 128],
                                 wp3r[:, m2 * 128:(m2 + 1) * 128],
                                 h2t[:, :], start=True, stop=True)
            g = singles.tile([128, 8], F32, tag="g", name="g")
            nc.vector.tensor_reduce(
                out=g[:, :], in_=h3_ps[:, :].rearrange("p (n x) -> p n x", n=8),
                axis=AX.X, op=ALU.max)
            gb = singles.tile([128, 8], F32, tag="gb", name="gb")
            nc.vector.tensor_add(gb[:, :], g[:, :], bp3t)
            nc.sync.dma_start(out=io["g3o"][:, :], in_=gb[:, :])


def _build():
    global _nc_cache
    if _nc_cache is not None:
        return _nc_cache
    nc = bacc.Bacc(trn_type="TRN2")

    io = dict(
        f1=nc.dram_tensor("f1", [C, N], F32, kind="ExternalInput"),
        packa1=nc.dram_tensor("packa1", [128, XA1], F32, kind="ExternalInput"),
        packa2=nc.dram_tensor("packa2", [128, XA2], F32, kind="ExternalInput"),
        packb=nc.dram_tensor("packb", [128, XB], F32, kind="ExternalInput"),
        wp3t=nc.dram_tensor("wp3t", [128, N], F32, kind="ExternalInput"),
        ffo=nc.dram_tensor("ffo", [3, JS], F32, kind="ExternalOutput"),
        pfo=nc.dram_tensor("pfo", [C, JS], F32, kind="ExternalOutput"),
        g3o=nc.dram_tensor("g3o", [128, 8], F32, kind="ExternalOutput"),
    )
    with tile.TileContext(nc) as tc:
        _emit(tc, nc, io)
    nc.finalize()
    _nc_cache = nc
    return nc


def _pack(table, fields):
    cols = sum(c for _, _, c in table)
    out = np.zeros((128, cols), np.float32)
    col = 0
    for name, rows, ncols in table:
        v = fields[name]
        assert v.shape == (rows, ncols), (name, v.shape, rows, ncols)
        out[:rows, col:col + ncols] = v
        col += ncols
    return out


def kernel(**inputs):
    global LAST_RESULT
    nc = _build()
    a = {k: np.ascontiguousarray(np.asarray(v, dtype=np.float32))
         for k, v in inputs.items()}
    c = np.ascontiguousarray

    pc_1 = a["pc_1"][0]          # [N, 3]
    F1 = a["pc_feat_1"][0]       # [C, N]
    F2 = a["pc_feat_2"][0]       # [C, N]
    W1_0, b1_0 = a["W1_0"], a["b1_0"]
    W1_1, b1_1 = a["W1_1"], a["b1_1"]
    W2_0, b2_0 = a["W2_0"], a["b2_0"]
    W2_1, b2_1 = a["W2_1"], a["b2_1"]
    Wp1, bp1 = a["Wp1"], a["bp1"]
    Wp2, bp2 = a["Wp2"], a["bp2"]
    Wp3, bp3 = a["Wp3"], a["bp3"]

    bdm = np.zeros((128, 128), np.float32)
    bdm[:C, :C] = W1_1.T
    bdm[C:, C:] = W1_1.T

    sharedA = dict(
        w10lt2=np.concatenate([W1_0[:, :C].T, W1_0[:, :C].T], axis=1),
        w10rt2=np.concatenate([W1_0[:, C:].T, W1_0[:, C:].T], axis=1),
        b10s=np.concatenate([b1_0, b1_0])[:, None],
        bd=bdm,
        b11s=np.concatenate([b1_1, b1_1])[:, None],
    )
    sharedB = dict(
        w20t=W2_0.T, w21t=W2_1.T, wp1t=Wp1.T, wp2t=Wp2.T,
        b20=b2_0[:, None], b21=b2_1[:, None], bp1c=bp1[:, None],
        bp2c=bp2[:, None], bp3t=bp3.reshape(8, 128).T,
    )
    wp3t = c(Wp3.T)

    in_maps = []
    for k in range(NCORES):
        sl = slice(k * JS, (k + 1) * JS)
        fa = dict(sharedA)
        fa["f2s"] = F2[:, sl]
        fb = dict(sharedB)
        fb["pc1b"] = pc_1[sl, :].T + b2_1[:, None]
        in_maps.append(dict(
            f1=F1,
            packa1=_pack(PACKA1, fa),
            packa2=_pack(PACKA2, fa),
            packb=_pack(PACKB, fb),
            wp3t=wp3t,
        ))

    res = run_bass_kernel_spmd(nc, in_maps, core_ids=list(range(NCORES)),
                               trace=TRACE)
    LAST_RESULT = res
    outs = res.results

    ff = np.concatenate([outs[k]["ffo"] for k in range(NCORES)], axis=1)[None]
    pf_all = np.concatenate([outs[k]["pfo"].T for k in range(NCORES)], axis=0)
    g = np.max(np.stack([outs[k]["g3o"].T.reshape(N) for k in range(NCORES)]),
               axis=0)
    fc = np.concatenate([np.broadcast_to(g, (N, N)), pf_all], axis=1)[None]
    return c(ff.astype(np.float32)), c(fc.astype(np.float32))
